# revision 17
# baseline (speedup 1.0000x reference)
"""Trainium2 Bass kernel for nn_Decoder (Bahdanau-attention LSTM decoder).

B=256,T=128,ENC=DEC=256,OUT=3. Data-parallel over batch: 8 cores x 32 batch.

v4 design (per core): two independent 16-batch groups software-pipelined
half a step apart, so one group's ScalarE tanh overlaps the other group's
back-half (softmax/ctx/gates/LSTM).  Emission order per step:
  BACK(g1, s-1), FRONT(g0, s), BACK(g0, s), FRONT(g1, s)
FRONT = z1, gates y/bias/h parts, bcast-add, tanh, scores, exp.
BACK  = E^T, diag, ctx, Dinv-scale, ctx^T, gates ctx part, LSTM.

Attention matmuls are 4-way column-packed (tile_position col-groups, 4
batches each).  Gates and LSTM are computed transposed (feature-major), so
no per-step state transposes are needed.  Sigmoid is computed via
tanh(x/2) identities (states stored doubled, consumers pre-scaled 0.5) so
ScalarE never leaves the exp/tanh table set.  PSUM co-tenancy packs each
group's step state into 3 banks (collision-safety via dependency chains).
"""

import sys
import numpy as np

sys.path.insert(0, "/opt/trn_rl_repo")

import ml_dtypes

BF16 = ml_dtypes.bfloat16

NCORES = 8
BL = 32          # batch per core
GB = 16          # batch per pipeline group
T = 128          # encoder positions == decoder steps
ENC = 256
DEC = 256
OUT = 3
BT = BL * T      # 4096
GT = GB * T      # 2048
S = 128          # decoder steps

_BUILT = None


def _build_nc():
    from contextlib import ExitStack
    from concourse import bacc, mybir, tile

    dt = mybir.dt
    F32, B16 = dt.float32, dt.bfloat16
    AF = mybir.ActivationFunctionType
    OP = mybir.AluOpType

    nc = bacc.Bacc("TRN2", target_bir_lowering=False, debug=False,
                   enable_asserts=False, num_devices=NCORES)

    di = lambda n, sh, d: nc.dram_tensor(n, sh, d, kind="ExternalInput").ap()
    xt = di("xt", [ENC, BT], B16)         # X^T, cols (g, t, b')
    x = di("x", [BT, ENC], B16)           # X, rows b*128+t
    yb = di("yb", [4, S * BL], B16)       # rows [y0,y1,y2,1], cols s*32+b
    w2t = di("w2t", [ENC, ENC], B16)
    w1t = di("w1t", [2 * DEC, ENC], B16)
    w3d = di("w3d", [128, 2048], B16)     # [f, fc*1024+b*32+((b%16)&3)]
    bc = di("bc", [ENC, 1], F32)
    wyb = di("wyb", [4, 4 * DEC], B16)
    wgc = di("wgc", [ENC, 4 * DEC], B16)
    wgh = di("wgh", [DEC, 4 * DEC], B16)
    fct = di("fct", [DEC + ENC, OUT], B16)
    fcb = di("fcb", [1, OUT], B16)
    onesr = di("onesr", [1, 512], B16)
    i128 = di("i128", [128, 128], F32)
    selm = di("selm", [128, 16], B16)     # col b'=4a+i selects row 32a+i
    o = nc.dram_tensor("o", [OUT, S * BL], dt.float32, kind="ExternalOutput").ap()

    with tile.TileContext(nc) as tc, ExitStack() as ctx:
        # ---------------- persistent SBUF ----------------
        P = ctx.enter_context(tc.tile_pool(name="persist", bufs=1))
        Z2 = P.tile([128, 2 * BT], B16, tag="z2", name="Z2")
        TIN = P.tile([128, 2 * BT], B16, tag="tin", name="TIN")
        TOUT = P.tile([128, 2 * BT], B16, tag="tout", name="TOUT")
        XS = P.tile([128, BL * ENC], B16, tag="xs")
        YBS = P.tile([4, S * BL], B16, tag="ybs")
        W1TS = P.tile([128, 4 * ENC], B16, tag="w1ts")
        W3DS = P.tile([128, 2048], B16, tag="w3ds")
        BCS = P.tile([128, 2], F32, tag="bcs")
        WYBS = P.tile([4, 4 * DEC], B16, tag="wybs")
        WGCS = P.tile([128, 2 * 4 * DEC], B16, tag="wgcs")
        WGHS = P.tile([128, 2 * 4 * DEC], B16, tag="wghs")
        FCTS = P.tile([128, 4 * OUT], B16, tag="fcts")
        FCBS = P.tile([1, OUT], B16, tag="fcbs")
        ONES = P.tile([1, 512], B16, tag="ones")
        I128 = P.tile([128, 128], F32, tag="i128")
        SELB = P.tile([128, 16], B16, tag="selb")
        TH = [P.tile([128, S * BL], B16, tag=f"th{i}", name=f"TH_{i}") for i in range(4)]
        DIAG = P.tile([128, BL * 32], B16, tag="diag")
        ZB16 = P.tile([128, 16], B16, tag="zb16")
        CF32 = [[P.tile([128, 32], F32, tag=f"cf{g}{i}", name=f"CF_{g}_{i}")
                 for i in range(2)] for g in range(2)]
        CB16 = [[P.tile([128, 32], B16, tag=f"cb{g}{i}", name=f"CB_{g}_{i}")
                 for i in range(2)] for g in range(2)]

        for b in range(BL):
            nc.sync.dma_start(XS[:, b * ENC:(b + 1) * ENC], x[b * T:(b + 1) * T, :])
        nc.sync.dma_start(YBS[:], yb[:])
        for kc in range(4):
            nc.sync.dma_start(W1TS[:, kc * ENC:(kc + 1) * ENC],
                              w1t[kc * 128:(kc + 1) * 128, :])
        nc.sync.dma_start(W3DS[:], w3d[:])
        for c in range(2):
            nc.sync.dma_start(BCS[:, c:c + 1], bc[c * 128:(c + 1) * 128, :])
        nc.sync.dma_start(WYBS[:], wyb[:])
        for j in range(2):
            nc.sync.dma_start(WGCS[:, j * 1024:(j + 1) * 1024],
                              wgc[j * 128:(j + 1) * 128, :])
            nc.sync.dma_start(WGHS[:, j * 1024:(j + 1) * 1024],
                              wgh[j * 128:(j + 1) * 128, :])
        for kc in range(4):
            nc.sync.dma_start(FCTS[:, kc * OUT:(kc + 1) * OUT],
                              fct[kc * 128:(kc + 1) * 128, :])
        nc.sync.dma_start(FCBS[:], fcb[:])
        nc.sync.dma_start(ONES[:], onesr[:])
        nc.sync.dma_start(I128[:], i128[:])
        nc.sync.dma_start(SELB[:], selm[:])

        nc.vector.memset(DIAG[:], 0.0)
        nc.vector.memset(ZB16[:], 0.0)
        for g in range(2):
            nc.vector.memset(CF32[g][0][:], 0.0)
            nc.vector.memset(CB16[g][0][:], 0.0)

        # ---------------- z2 precompute (bias folded in) ----------------
        with tc.tile_pool(name="xts", bufs=1) as xtp, \
             tc.tile_pool(name="z2ps", bufs=2, space="PSUM") as z2ps, \
             tc.tile_pool(name="w2p", bufs=1) as w2p:
            W2TS = w2p.tile([128, 2 * ENC], B16)
            for ec in range(2):
                nc.sync.dma_start(W2TS[:, ec * ENC:(ec + 1) * ENC],
                                  w2t[ec * 128:(ec + 1) * 128, :])
            XTS = [xtp.tile([128, BT], B16, tag=f"xt{e}", name=f"XTS_{e}")
                   for e in range(2)]
            for ec in range(2):
                nc.sync.dma_start(XTS[ec][:], xt[ec * 128:(ec + 1) * 128, :])
            for fc in range(2):
                for nq in range(8):
                    zp = z2ps.tile([128, 512], F32, tag="zp")
                    for ec in range(2):
                        nc.tensor.matmul(
                            zp[:], W2TS[:, ec * ENC + fc * 128: ec * ENC + fc * 128 + 128],
                            XTS[ec][:, nq * 512:(nq + 1) * 512],
                            start=(ec == 0), stop=(ec == 1))
                    dst0 = (nq // 4) * 2 * GT + fc * GT + (nq % 4) * 512
                    nc.scalar.activation(Z2[:, dst0:dst0 + 512], zp[:],
                                         AF.Identity, bias=BCS[:, fc:fc + 1])

        # ---------------- step pools ----------------
        loop_ctx = ExitStack()
        sb_p = loop_ctx.enter_context(tc.tile_pool(name="small", bufs=2))
        st_p = loop_ctx.enter_context(tc.tile_pool(name="state", bufs=2))
        # per-group psum co-tenant tiles, bufs=1 (3 banks per group)
        scz_p = [loop_ctx.enter_context(tc.tile_pool(name=f"scz{g}", bufs=1, space="PSUM"))
                 for g in range(2)]
        tp_p = [loop_ctx.enter_context(tc.tile_pool(name=f"tp{g}", bufs=1, space="PSUM"))
                for g in range(2)]
        cx_p = [loop_ctx.enter_context(tc.tile_pool(name=f"cx{g}", bufs=1, space="PSUM"))
                for g in range(2)]
        gp_p = [loop_ctx.enter_context(tc.tile_pool(name=f"gp{g}", bufs=1, space="PSUM"))
                for g in range(2)]
        SCZ = [scz_p[g].tile([128, 160], F32, tag=f"scz{g}", name=f"SCZ_{g}")
               for g in range(2)]
        TP = [tp_p[g].tile([128, 48], F32, tag=f"tp{g}", name=f"TP_{g}")
              for g in range(2)]
        CX = [cx_p[g].tile([128, 256], F32, tag=f"cx{g}", name=f"CX_{g}")
              for g in range(2)]
        GP = [gp_p[g].tile([128, 128], F32, tag=f"gp{g}", name=f"GP_{g}")
              for g in range(2)]

        mm = nc.tensor.matmul
        # per-group live sbuf tiles produced by FRONT, consumed by BACK
        live = [{}, {}]

        def fa_front(g, s):
            c0 = s * BL + g * GB
            hprev = ([ZB16[:], ZB16[:]] if s == 0 else
                     [TH[0][:, c0 - BL: c0 - BL + GB], TH[1][:, c0 - BL: c0 - BL + GB]])
            live[g]["hprev"] = hprev
            cb = CB16[g][s % 2]

            # z1 = W1 @ hc  [128, (fc,b')]  -> SCZ cols 0:32
            for fc in range(2):
                for kc in range(4):
                    rhs = (hprev[kc] if kc < 2
                           else (ZB16[:] if s == 0
                                 else cb[:, (kc - 2) * 16:(kc - 1) * 16]))
                    mm(SCZ[g][:, fc * 16:(fc + 1) * 16],
                       W1TS[:, kc * ENC + fc * 128: kc * ENC + fc * 128 + 128],
                       rhs, start=(fc == 0 and kc == 0), stop=(fc == 1 and kc == 3),
                       skip_group_check=True)
            z1s = sb_p.tile([128, 32], B16, tag=f"z1s{g}")
            nc.vector.tensor_copy(z1s[:], SCZ[g][:, 0:32])

            # bcast add; fc1 is split DVE/GpSimd halves to free DVE time
            fsl0 = slice(g * 2 * GT, g * 2 * GT + GT)
            fsl1 = slice(g * 2 * GT + GT, g * 2 * GT + 2 * GT)
            tin0 = TIN[:, fsl0].rearrange("p (t b) -> p t b", b=GB)
            z20 = Z2[:, fsl0].rearrange("p (t b) -> p t b", b=GB)
            z1b0 = z1s[:, None, 0:16].broadcast_to([128, T, GB])
            nc.vector.tensor_add(tin0, z20, z1b0)
            tin1 = TIN[:, fsl1].rearrange("p (t b) -> p t b", b=GB)
            z21 = Z2[:, fsl1].rearrange("p (t b) -> p t b", b=GB)
            z1b1 = z1s[:, None, 16:32].broadcast_to([128, T, GB])
            nc.vector.tensor_add(tin1, z21, z1b1)

        def fa_tanh(g, s):
            for fc in range(2):
                fsl = slice(g * 2 * GT + fc * GT, g * 2 * GT + (fc + 1) * GT)
                nc.scalar.activation(TOUT[:, fsl], TIN[:, fsl], AF.Tanh)

        def gates_yh(s):
            # per-group y+h gate parts
            for g in range(2):
                c0 = s * BL + g * GB
                for j in range(8):
                    mm(GP[g][:, j * 16:(j + 1) * 16],
                       WYBS[:, j * 128:(j + 1) * 128],
                       YBS[:, c0:c0 + GB],
                       start=(j == 0), stop=False, skip_group_check=True)
                if s > 0:
                    for half in range(2):
                        for j in range(8):
                            mm(GP[g][:, j * 16:(j + 1) * 16],
                               WGHS[:, half * 1024 + j * 128: half * 1024 + (j + 1) * 128],
                               TH[half][:, c0 - BL: c0 - BL + GB],
                               start=False, stop=False, skip_group_check=True)

        def fb(g, s):
            # scores: diag-lhsT, 4-way column-packed; psum row 32*cg+i = b'=4cg+i
            for fc in range(2):
                t3 = TOUT[:, g * 2 * GT + fc * GT:
                          g * 2 * GT + (fc + 1) * GT].rearrange(
                    "p (t b) -> p t b", b=GB)
                for i in range(4):
                    for cg in range(4):
                        bp = 4 * cg + i
                        b = g * GB + bp
                        mm(SCZ[g][32 * cg:32 * (cg + 1), 32:160],
                           W3DS[:, fc * 1024 + b * 32: fc * 1024 + b * 32 + 32],
                           t3[:, :, bp],
                           start=(fc == 0 and i == 0),
                           stop=(fc == 1 and i == 3),
                           tile_position=(0, 32 * cg), skip_group_check=True)

            E = sb_p.tile([128, 128], B16, tag=f"E{g}")
            nc.scalar.activation(E[:], SCZ[g][:, 32:160], AF.Exp)
            D = sb_p.tile([128, 1], F32, tag=f"D{g}")
            nc.vector.tensor_reduce(D[:], E[:], mybir.AxisListType.X, OP.add)
            Dinv = sb_p.tile([128, 1], F32, tag=f"Di{g}")
            nc.vector.reciprocal(Dinv[:], D[:])
            live[g]["E"] = E
            live[g]["Dinv"] = Dinv

        def back(g, s):
            last = (s == S - 1)
            c0 = s * BL + g * GB
            E, Dinv = live[g]["E"], live[g]["Dinv"]

            # E^T selected columns via tiny matmul: out[t, b'] = E[slot(b'), t]
            mm(TP[g][:, 0:16], E[:], SELB[:], start=True, stop=True,
               skip_group_check=True)
            dg3 = DIAG[:, g * 512:(g + 1) * 512].rearrange("p (a c) -> p a c", c=128)
            et3 = TP[g][:, 0:16].rearrange("p (a c) -> p a c", c=4)
            nc.vector.tensor_copy(dg3[:, :, 0:100:33], et3[:])

            for i in range(4):
                for cg in range(4):
                    b = g * GB + 4 * cg + i
                    mm(CX[g][32 * cg:32 * (cg + 1), :],
                       DIAG[:, b * 32:(b + 1) * 32],
                       XS[:, b * ENC:(b + 1) * ENC],
                       start=(i == 0), stop=(i == 3),
                       tile_position=(0, 32 * cg), skip_group_check=True)
            cxs = sb_p.tile([128, ENC], B16, tag=f"cxs{g}")
            nc.vector.tensor_scalar_mul(cxs[:], CX[g][:], Dinv[:])

            for half in range(2):
                mm(TP[g][:, 16 + half * 16: 32 + half * 16],
                   cxs[:, half * 128:(half + 1) * 128], SELB[:],
                   start=True, stop=True, skip_group_check=True)
            for half in range(2):
                nc.vector.tensor_copy(TH[2 + half][:, c0:c0 + GB],
                                      TP[g][:, 16 + half * 16: 32 + half * 16])

            if last:
                for j in range(2):
                    nc.vector.tensor_copy(TH[j][:, c0:c0 + GB],
                                          TH[j][:, c0 - BL: c0 - BL + GB])
                return

            for half in range(2):
                for j in range(8):
                    mm(GP[g][:, j * 16:(j + 1) * 16],
                       WGCS[:, half * 1024 + j * 128: half * 1024 + (j + 1) * 128],
                       TH[2 + half][:, c0:c0 + GB],
                       start=False, stop=(half == 1 and j == 7),
                       skip_group_check=True)

            # LSTM elementwise (tau-form), [128, 32] = (dchunk, b')
            cf = CF32[g][s % 2]
            # one ACT call: tau for i,f,o and tanh(g) (g-weights doubled on host)
            sifg = st_p.tile([128, 128], F32, tag=f"sif{g}")
            nc.scalar.activation(sifg[:], GP[g][:], AF.Tanh, scale=0.5)
            sif = sifg[:, 0:96]
            tg = sifg[:, 96:128]
            t1 = st_p.tile([128, 32], F32, tag=f"t1{g}")
            nc.vector.scalar_tensor_tensor(t1[:], sifg[:, 32:64], 1.0, cf[:],
                                           OP.add, OP.mult)
            t2 = st_p.tile([128, 32], F32, tag=f"t2{g}")
            nc.vector.scalar_tensor_tensor(t2[:], sifg[:, 0:32], 1.0, tg,
                                           OP.add, OP.mult)
            cn = CF32[g][(s + 1) % 2]
            nc.vector.scalar_tensor_tensor(cn[:], t1[:], 0.5, t2[:],
                                           OP.mult, OP.add)
            tc_ = st_p.tile([128, 32], F32, tag=f"tc{g}")
            nc.scalar.activation(tc_[:], cn[:], AF.Tanh, scale=0.5)
            for j in range(2):
                nc.vector.scalar_tensor_tensor(
                    TH[j][:, c0:c0 + GB], sifg[:, 64 + j * 16: 80 + j * 16],
                    1.0, tc_[:, j * 16:(j + 1) * 16], OP.add, OP.mult)
            nc.vector.tensor_copy(CB16[g][(s + 1) % 2][:], cn[:])

        # staggered pipeline, emission ordered to avoid head-of-line blocks:
        # ACT queue/step: tanh-g0 x2, sifg/tc-g1(s-1), tanh-g1 x2, exp-g0,
        #                 sifg/tc-g0, exp-g1
        for s in range(S):
            fa_front(0, s)
            fa_tanh(0, s)
            if s > 0:
                back(1, s - 1)
            fa_front(1, s)
            fb(0, s)
            if s < S - 1:
                gates_yh(s)
            fa_tanh(1, s)
            back(0, s)
            fb(1, s)
        back(1, S - 1)

        loop_ctx.close()

        # ---------------- output head ----------------
        with tc.tile_pool(name="ops", bufs=2, space="PSUM") as ops, \
             tc.tile_pool(name="ost", bufs=2) as ost:
            for nq in range(8):
                op = ops.tile([OUT, 512], F32, tag="op")
                for kc in range(4):
                    mm(op[:], FCTS[:, kc * OUT:(kc + 1) * OUT],
                       TH[kc][:, nq * 512:(nq + 1) * 512],
                       start=(kc == 0), stop=False)
                mm(op[:], FCBS[:], ONES[:], start=False, stop=True)
                ot = ost.tile([OUT, 512], F32, tag="ot")
                nc.vector.tensor_copy(ot[:], op[:])
                nc.sync.dma_start(o[:, nq * 512:(nq + 1) * 512], ot[:])

    nc.compile()
    return nc


def _host_prep(inputs):
    f32 = np.float32
    ie = np.asarray(inputs["input_encoded"], f32)
    ys = np.asarray(inputs["y_seq"], f32)
    a1w = np.asarray(inputs["attn1_w"], f32)
    a1b = np.asarray(inputs["attn1_b"], f32)
    a2w = np.asarray(inputs["attn2_w"], f32)
    a2b = np.asarray(inputs["attn2_b"], f32)
    a3w = np.asarray(inputs["attn3_w"], f32)
    tw = np.asarray(inputs["tilde_w"], f32)
    tb = np.asarray(inputs["tilde_b"], f32)
    wih = np.asarray(inputs["w_ih"], f32)
    whh = np.asarray(inputs["w_hh"], f32)
    bih = np.asarray(inputs["b_ih"], f32)
    bhh = np.asarray(inputs["b_hh"], f32)
    f1w = np.asarray(inputs["fc1_w"], f32)
    f1b = np.asarray(inputs["fc1_b"], f32)
    f2w = np.asarray(inputs["fc2_w"], f32)
    f2b = np.asarray(inputs["fc2_b"], f32)

    wcomb = wih @ tw
    gbias = wih @ tb + bih + bhh
    fc = f2w @ f1w
    fcbias = f2w @ f1b + f2b

    perm = np.concatenate([np.arange(0, 512),
                           np.arange(768, 1024),
                           np.arange(512, 768)])
    wcombT = np.ascontiguousarray(wcomb.T)[:, perm]
    whhT = np.ascontiguousarray(whh.T)[:, perm]
    gbias_p = gbias[perm]
    wyb = np.concatenate([wcombT[0:3], gbias_p[None, :]], axis=0)

    # w3 diag arena: column ((b%16)&3) within each b's 32-col slice
    w3diag = np.zeros((128, 2, 32, 32), f32)
    for fc_ in range(2):
        for b in range(32):
            w3diag[:, fc_, b, (b % 16) & 3] = a3w[0, fc_ * 128:(fc_ + 1) * 128]
    w3diag = w3diag.reshape(128, 2048)

    # double the g-gate columns so one tanh(x/2) ACT call serves i,f,o,g
    wcombT[:, 768:1024] *= 2.0
    whhT[:, 768:1024] *= 2.0
    gbias_p = gbias_p.copy()
    gbias_p[768:1024] *= 2.0
    wyb = np.concatenate([wcombT[0:3], gbias_p[None, :]], axis=0)

    selm = np.zeros((128, 16), np.float32)
    for bp in range(16):
        selm[32 * (bp >> 2) + (bp & 3), bp] = 1.0

    fcT = np.ascontiguousarray(fc.T).copy()
    fcT[0:256] *= 0.5
    shared = {
        "w2t": np.ascontiguousarray(a2w.T).astype(BF16),
        "w1t": (np.ascontiguousarray(a1w.T) * 0.5).astype(BF16),
        "w3d": w3diag.astype(BF16),
        "bc": (a1b + a2b)[:, None].astype(f32),
        "wyb": wyb.astype(BF16),
        "wgc": np.ascontiguousarray(wcombT[3:259]).astype(BF16),
        "wgh": (whhT * 0.5).astype(BF16),
        "fct": fcT.astype(BF16),
        "fcb": fcbias[None, :].astype(BF16),
        "onesr": np.ones((1, 512), BF16),
        "i128": np.eye(128, dtype=f32),
        "selm": selm.astype(BF16),
    }
    in_maps = []
    for i in range(NCORES):
        b0 = i * BL
        xe = ie[b0:b0 + BL]                            # [32,128,256]
        m = dict(shared)
        # xt cols ordered (g, t, b'): group-contiguous 2048-col halves
        m["xt"] = np.ascontiguousarray(
            xe.reshape(2, GB, T, ENC).transpose(3, 0, 2, 1).reshape(ENC, BT)
        ).astype(BF16)
        m["x"] = xe.reshape(BT, ENC).astype(BF16)
        yt = ys[b0:b0 + BL].transpose(2, 1, 0).reshape(OUT, S * BL)
        m["yb"] = np.concatenate(
            [yt, np.ones((1, S * BL), f32)], axis=0).astype(BF16)
        in_maps.append(m)
    return in_maps


def kernel(**inputs):
    global _BUILT
    from concourse import bass_utils
    if _BUILT is None:
        _BUILT = _build_nc()
    nc = _BUILT
    import os
    in_maps = _host_prep(inputs)
    trace = bool(int(os.environ.get("KERNEL_TRACE", "0")))
    res = bass_utils.run_bass_kernel_spmd(nc, in_maps, core_ids=list(range(NCORES)),
                                          trace=trace)
    if trace:
        print(f"HW exec time: {res.exec_time_ns} ns  (mean {res.mean_exec_time_ns})")
        globals()['_LAST_RESULTS'] = res
    outs = []
    for i in range(NCORES):
        oc = res.results[i]["o"]                       # [3, 4096] (j, s*32+b)
        outs.append(oc.reshape(OUT, S, BL).transpose(2, 1, 0))
    return np.concatenate(outs, axis=0).astype(np.float32)


if __name__ == "__main__":
    pass



# revision 25
# speedup vs baseline: 1.0159x; 1.0159x over previous
"""Trainium2 Bass kernel for nn_Decoder (Bahdanau-attention LSTM decoder).

B=256,T=128,ENC=DEC=256,OUT=3. Data-parallel over batch: 8 cores x 32 batch.

v4 design (per core): two independent 16-batch groups software-pipelined
half a step apart, so one group's ScalarE tanh overlaps the other group's
back-half (softmax/ctx/gates/LSTM).  Emission order per step:
  BACK(g1, s-1), FRONT(g0, s), BACK(g0, s), FRONT(g1, s)
FRONT = z1, gates y/bias/h parts, bcast-add, tanh, scores, exp.
BACK  = E^T, diag, ctx, Dinv-scale, ctx^T, gates ctx part, LSTM.

Attention matmuls are 4-way column-packed (tile_position col-groups, 4
batches each).  Gates and LSTM are computed transposed (feature-major), so
no per-step state transposes are needed.  Sigmoid is computed via
tanh(x/2) identities (states stored doubled, consumers pre-scaled 0.5) so
ScalarE never leaves the exp/tanh table set.  PSUM co-tenancy packs each
group's step state into 3 banks (collision-safety via dependency chains).
"""

import sys
import numpy as np

sys.path.insert(0, "/opt/trn_rl_repo")

import ml_dtypes

BF16 = ml_dtypes.bfloat16

NCORES = 8
BL = 32          # batch per core
GB = 16          # batch per pipeline group
T = 128          # encoder positions == decoder steps
ENC = 256
DEC = 256
OUT = 3
BT = BL * T      # 4096
GT = GB * T      # 2048
S = 128          # decoder steps

_BUILT = None


def _build_nc():
    from contextlib import ExitStack
    from concourse import bacc, mybir, tile

    dt = mybir.dt
    F32, B16 = dt.float32, dt.bfloat16
    AF = mybir.ActivationFunctionType
    OP = mybir.AluOpType

    nc = bacc.Bacc("TRN2", target_bir_lowering=False, debug=False,
                   enable_asserts=False, num_devices=NCORES)

    di = lambda n, sh, d: nc.dram_tensor(n, sh, d, kind="ExternalInput").ap()
    xt = di("xt", [ENC, BT], B16)         # X^T, cols (g, t, b')
    x = di("x", [BT, ENC], B16)           # X, rows b*128+t
    yb = di("yb", [4, S * BL], B16)       # rows [y0,y1,y2,1], cols s*32+b
    w2t = di("w2t", [ENC, ENC], B16)
    w1t = di("w1t", [2 * DEC, ENC], B16)
    w3d = di("w3d", [128, 2048], B16)     # [f, fc*1024+b*32+((b%16)&3)]
    bc = di("bc", [ENC, 1], F32)
    wyb = di("wyb", [4, 4 * DEC], B16)
    wgc = di("wgc", [ENC, 4 * DEC], B16)
    wgh = di("wgh", [DEC, 4 * DEC], B16)
    fct = di("fct", [DEC + ENC, OUT], B16)
    fcb = di("fcb", [1, OUT], B16)
    onesr = di("onesr", [1, 512], B16)
    i128 = di("i128", [128, 128], F32)
    selm = di("selm", [128, 16], B16)     # col b'=4a+i selects row 32a+i
    o = nc.dram_tensor("o", [OUT, S * BL], dt.float32, kind="ExternalOutput").ap()

    with tile.TileContext(nc) as tc, ExitStack() as ctx:
        # ---------------- persistent SBUF ----------------
        P = ctx.enter_context(tc.tile_pool(name="persist", bufs=1))
        Z2 = P.tile([128, 2 * BT], B16, tag="z2", name="Z2")
        TIN = P.tile([128, 2 * BT], B16, tag="tin", name="TIN")
        TOUT = P.tile([128, 2 * BT], B16, tag="tout", name="TOUT")
        XS = P.tile([128, BL * ENC], B16, tag="xs")
        YBS = P.tile([4, S * BL], B16, tag="ybs")
        W1TS = P.tile([128, 4 * ENC], B16, tag="w1ts")
        W3DS = P.tile([128, 2048], B16, tag="w3ds")
        BCS = P.tile([128, 2], F32, tag="bcs")
        WYBS = P.tile([4, 4 * DEC], B16, tag="wybs")
        WGCS = P.tile([128, 2 * 4 * DEC], B16, tag="wgcs")
        WGHS = P.tile([128, 2 * 4 * DEC], B16, tag="wghs")
        FCTS = P.tile([128, 4 * OUT], B16, tag="fcts")
        FCBS = P.tile([1, OUT], B16, tag="fcbs")
        ONES = P.tile([1, 512], B16, tag="ones")
        I128 = P.tile([128, 128], F32, tag="i128")
        SELB = P.tile([128, 16], B16, tag="selb")
        TH = [P.tile([128, S * BL], B16, tag=f"th{i}", name=f"TH_{i}") for i in range(4)]
        DIAG = P.tile([128, BL * 32], B16, tag="diag")
        ZB16 = P.tile([128, 16], B16, tag="zb16")
        CF32 = [[P.tile([128, 32], F32, tag=f"cf{g}{i}", name=f"CF_{g}_{i}")
                 for i in range(2)] for g in range(2)]
        CB16 = [[P.tile([128, 32], B16, tag=f"cb{g}{i}", name=f"CB_{g}_{i}")
                 for i in range(2)] for g in range(2)]

        for b in range(BL):
            nc.sync.dma_start(XS[:, b * ENC:(b + 1) * ENC], x[b * T:(b + 1) * T, :])
        nc.sync.dma_start(YBS[:], yb[:])
        for kc in range(4):
            nc.sync.dma_start(W1TS[:, kc * ENC:(kc + 1) * ENC],
                              w1t[kc * 128:(kc + 1) * 128, :])
        nc.sync.dma_start(W3DS[:], w3d[:])
        for c in range(2):
            nc.sync.dma_start(BCS[:, c:c + 1], bc[c * 128:(c + 1) * 128, :])
        nc.sync.dma_start(WYBS[:], wyb[:])
        for j in range(2):
            nc.sync.dma_start(WGCS[:, j * 1024:(j + 1) * 1024],
                              wgc[j * 128:(j + 1) * 128, :])
            nc.sync.dma_start(WGHS[:, j * 1024:(j + 1) * 1024],
                              wgh[j * 128:(j + 1) * 128, :])
        for kc in range(4):
            nc.sync.dma_start(FCTS[:, kc * OUT:(kc + 1) * OUT],
                              fct[kc * 128:(kc + 1) * 128, :])
        nc.sync.dma_start(FCBS[:], fcb[:])
        nc.sync.dma_start(ONES[:], onesr[:])
        nc.sync.dma_start(I128[:], i128[:])
        nc.sync.dma_start(SELB[:], selm[:])

        nc.vector.memset(DIAG[:], 0.0)
        nc.vector.memset(ZB16[:], 0.0)
        for g in range(2):
            nc.vector.memset(CF32[g][0][:], 0.0)
            nc.vector.memset(CB16[g][0][:], 0.0)

        # ---------------- z2 precompute (bias folded in) ----------------
        with tc.tile_pool(name="xts", bufs=1) as xtp, \
             tc.tile_pool(name="z2ps", bufs=2, space="PSUM") as z2ps, \
             tc.tile_pool(name="w2p", bufs=1) as w2p:
            W2TS = w2p.tile([128, 2 * ENC], B16)
            for ec in range(2):
                nc.sync.dma_start(W2TS[:, ec * ENC:(ec + 1) * ENC],
                                  w2t[ec * 128:(ec + 1) * 128, :])
            XTS = [xtp.tile([128, BT], B16, tag=f"xt{e}", name=f"XTS_{e}")
                   for e in range(2)]
            for ec in range(2):
                nc.sync.dma_start(XTS[ec][:], xt[ec * 128:(ec + 1) * 128, :])
            for fc in range(2):
                for nq in range(8):
                    zp = z2ps.tile([128, 512], F32, tag="zp")
                    for ec in range(2):
                        nc.tensor.matmul(
                            zp[:], W2TS[:, ec * ENC + fc * 128: ec * ENC + fc * 128 + 128],
                            XTS[ec][:, nq * 512:(nq + 1) * 512],
                            start=(ec == 0), stop=(ec == 1))
                    dst0 = (nq // 4) * 2 * GT + fc * GT + (nq % 4) * 512
                    nc.scalar.activation(Z2[:, dst0:dst0 + 512], zp[:],
                                         AF.Identity, bias=BCS[:, fc:fc + 1])

        # ---------------- step pools ----------------
        loop_ctx = ExitStack()
        sb_p = loop_ctx.enter_context(tc.tile_pool(name="small", bufs=2))
        st_p = loop_ctx.enter_context(tc.tile_pool(name="state", bufs=2))
        # per-group psum co-tenant tiles, bufs=1 (3 banks per group)
        scz_p = [loop_ctx.enter_context(tc.tile_pool(name=f"scz{g}", bufs=1, space="PSUM"))
                 for g in range(2)]
        tp_p = [loop_ctx.enter_context(tc.tile_pool(name=f"tp{g}", bufs=1, space="PSUM"))
                for g in range(2)]
        cx_p = [loop_ctx.enter_context(tc.tile_pool(name=f"cx{g}", bufs=1, space="PSUM"))
                for g in range(2)]
        gp_p = loop_ctx.enter_context(tc.tile_pool(name="gpm", bufs=2, space="PSUM"))
        SCZ = [scz_p[g].tile([128, 160], F32, tag=f"scz{g}", name=f"SCZ_{g}")
               for g in range(2)]
        TP = [tp_p[g].tile([128, 48], F32, tag=f"tp{g}", name=f"TP_{g}")
              for g in range(2)]
        CX = [cx_p[g].tile([128, 256], F32, tag=f"cx{g}", name=f"CX_{g}")
              for g in range(2)]
        # merged gates psum, double-buffered per step: cols (j 8, b 32)
        gpm_live = {}

        mm = nc.tensor.matmul
        # per-group live sbuf tiles produced by FRONT, consumed by BACK
        live = [{}, {}]

        def fa_front(g, s):
            c0 = s * BL + g * GB
            hprev = ([ZB16[:], ZB16[:]] if s == 0 else
                     [TH[0][:, c0 - BL: c0 - BL + GB], TH[1][:, c0 - BL: c0 - BL + GB]])
            live[g]["hprev"] = hprev
            cb = CB16[g][s % 2]

            # z1 = W1 @ hc  [128, (fc,b')]  -> SCZ cols 0:32
            for fc in range(2):
                for kc in range(4):
                    rhs = (hprev[kc] if kc < 2
                           else (ZB16[:] if s == 0
                                 else cb[:, (kc - 2) * 16:(kc - 1) * 16]))
                    mm(SCZ[g][:, fc * 16:(fc + 1) * 16],
                       W1TS[:, kc * ENC + fc * 128: kc * ENC + fc * 128 + 128],
                       rhs, start=(fc == 0 and kc == 0), stop=(fc == 1 and kc == 3),
                       skip_group_check=True)
            z1s = sb_p.tile([128, 32], B16, tag=f"z1s{g}")
            nc.vector.tensor_copy(z1s[:], SCZ[g][:, 0:32])

            # bcast add; fc1 is split DVE/GpSimd halves to free DVE time
            fsl0 = slice(g * 2 * GT, g * 2 * GT + GT)
            fsl1 = slice(g * 2 * GT + GT, g * 2 * GT + 2 * GT)
            tin0 = TIN[:, fsl0].rearrange("p (t b) -> p t b", b=GB)
            z20 = Z2[:, fsl0].rearrange("p (t b) -> p t b", b=GB)
            z1b0 = z1s[:, None, 0:16].broadcast_to([128, T, GB])
            nc.vector.tensor_add(tin0, z20, z1b0)
            tin1 = TIN[:, fsl1].rearrange("p (t b) -> p t b", b=GB)
            z21 = Z2[:, fsl1].rearrange("p (t b) -> p t b", b=GB)
            z1b1 = z1s[:, None, 16:32].broadcast_to([128, T, GB])
            nc.vector.tensor_add(tin1, z21, z1b1)

        def fa_tanh(g, s):
            for fc in range(2):
                fsl = slice(g * 2 * GT + fc * GT, g * 2 * GT + (fc + 1) * GT)
                nc.scalar.activation(TOUT[:, fsl], TIN[:, fsl], AF.Tanh)

        def gates_yh(s):
            # merged y+h gate parts for BOTH groups into a fresh (double-
            # buffered) psum tile; new instance per step kills the WAR race
            # with the previous step's strided sifg reads.
            GPM = gp_p.tile([128, 512], F32, tag="gpm", name="GPM")
            gpm_live[s] = GPM
            gpm_live.pop(s - 2, None)
            c0 = s * BL
            for j in range(8):
                mm(GPM[:, j * 32:(j + 1) * 32],
                   WYBS[:, j * 128:(j + 1) * 128],
                   YBS[:, c0:c0 + BL],
                   start=(j == 0), stop=False, skip_group_check=True)
            if s > 0:
                for half in range(2):
                    for j in range(8):
                        mm(GPM[:, j * 32:(j + 1) * 32],
                           WGHS[:, half * 1024 + j * 128: half * 1024 + (j + 1) * 128],
                           TH[half][:, c0 - BL: c0],
                           start=False, stop=False, skip_group_check=True)

        def fb(g, s):
            # scores: diag-lhsT, 4-way column-packed; psum row 32*cg+i = b'=4cg+i
            for fc in range(2):
                t3 = TOUT[:, g * 2 * GT + fc * GT:
                          g * 2 * GT + (fc + 1) * GT].rearrange(
                    "p (t b) -> p t b", b=GB)
                for i in range(4):
                    for cg in range(4):
                        bp = 4 * cg + i
                        b = g * GB + bp
                        mm(SCZ[g][32 * cg:32 * (cg + 1), 32:160],
                           W3DS[:, fc * 1024 + b * 32: fc * 1024 + b * 32 + 32],
                           t3[:, :, bp],
                           start=(fc == 0 and i == 0),
                           stop=(fc == 1 and i == 3),
                           tile_position=(0, 32 * cg), skip_group_check=True)

            E = sb_p.tile([128, 128], B16, tag=f"E{g}")
            nc.scalar.activation(E[:], SCZ[g][:, 32:160], AF.Exp)
            D = sb_p.tile([128, 1], F32, tag=f"D{g}")
            nc.vector.tensor_reduce(D[:], E[:], mybir.AxisListType.X, OP.add)
            Dinv = sb_p.tile([128, 1], F32, tag=f"Di{g}")
            nc.vector.reciprocal(Dinv[:], D[:])
            live[g]["E"] = E
            live[g]["Dinv"] = Dinv

        def back(g, s):
            last = (s == S - 1)
            c0 = s * BL + g * GB
            E, Dinv = live[g]["E"], live[g]["Dinv"]

            # E^T selected columns via tiny matmul: out[t, b'] = E[slot(b'), t]
            mm(TP[g][:, 0:16], E[:], SELB[:], start=True, stop=True,
               skip_group_check=True)
            dg3 = DIAG[:, g * 512:(g + 1) * 512].rearrange("p (a c) -> p a c", c=128)
            et3 = TP[g][:, 0:16].rearrange("p (a c) -> p a c", c=4)
            nc.vector.tensor_copy(dg3[:, :, 0:100:33], et3[:])

            for i in range(4):
                for cg in range(4):
                    b = g * GB + 4 * cg + i
                    mm(CX[g][32 * cg:32 * (cg + 1), :],
                       DIAG[:, b * 32:(b + 1) * 32],
                       XS[:, b * ENC:(b + 1) * ENC],
                       start=(i == 0), stop=(i == 3),
                       tile_position=(0, 32 * cg), skip_group_check=True)
            cxs = sb_p.tile([128, ENC], B16, tag=f"cxs{g}")
            nc.vector.tensor_scalar_mul(cxs[:], CX[g][:], Dinv[:])

            for half in range(2):
                mm(TP[g][:, 16 + half * 16: 32 + half * 16],
                   cxs[:, half * 128:(half + 1) * 128], SELB[:],
                   start=True, stop=True, skip_group_check=True)
            for half in range(2):
                nc.vector.tensor_copy(TH[2 + half][:, c0:c0 + GB],
                                      TP[g][:, 16 + half * 16: 32 + half * 16])

            if last:
                for j in range(2):
                    nc.vector.tensor_copy(TH[j][:, c0:c0 + GB],
                                          TH[j][:, c0 - BL: c0 - BL + GB])
                return

            GPM = gpm_live[s]
            for half in range(2):
                for j in range(8):
                    mm(GPM[:, j * 32 + g * 16: j * 32 + g * 16 + 16],
                       WGCS[:, half * 1024 + j * 128: half * 1024 + (j + 1) * 128],
                       TH[2 + half][:, c0:c0 + GB],
                       start=False, stop=(half == 1 and j == 7),
                       skip_group_check=True)

            # LSTM elementwise (tau-form), [128, 32] = (dchunk, b')
            cf = CF32[g][s % 2]
            # one ACT call: tau for i,f,o and tanh(g) (g-weights doubled on host)
            gv = GPM[:, 0:256].rearrange("p (j b) -> p j b", b=32)[:, :, g * 16:(g + 1) * 16]
            sifg = st_p.tile([128, 128], F32, tag=f"sif{g}")
            nc.scalar.activation(sifg[:], gv, AF.Tanh, scale=0.5)
            sif = sifg[:, 0:96]
            tg = sifg[:, 96:128]
            t1 = st_p.tile([128, 32], F32, tag=f"t1{g}")
            nc.vector.scalar_tensor_tensor(t1[:], sifg[:, 32:64], 1.0, cf[:],
                                           OP.add, OP.mult)
            t2 = st_p.tile([128, 32], F32, tag=f"t2{g}")
            nc.vector.scalar_tensor_tensor(t2[:], sifg[:, 0:32], 1.0, tg,
                                           OP.add, OP.mult)
            cn = CF32[g][(s + 1) % 2]
            nc.vector.scalar_tensor_tensor(cn[:], t1[:], 0.5, t2[:],
                                           OP.mult, OP.add)
            tc_ = st_p.tile([128, 32], F32, tag=f"tc{g}")
            nc.scalar.activation(tc_[:], cn[:], AF.Tanh, scale=0.5)
            for j in range(2):
                nc.vector.scalar_tensor_tensor(
                    TH[j][:, c0:c0 + GB], sifg[:, 64 + j * 16: 80 + j * 16],
                    1.0, tc_[:, j * 16:(j + 1) * 16], OP.add, OP.mult)
            nc.vector.tensor_copy(CB16[g][(s + 1) % 2][:], cn[:])

        # staggered pipeline, emission ordered to avoid head-of-line blocks:
        # ACT queue/step: tanh-g0 x2, sifg/tc-g1(s-1), tanh-g1 x2, exp-g0,
        #                 sifg/tc-g0, exp-g1
        fa_front(0, 0)
        for s in range(S):
            fa_tanh(0, s)
            if s > 0:
                back(1, s - 1)
            fa_front(1, s)
            fb(0, s)
            if s < S - 1:
                gates_yh(s)
            fa_tanh(1, s)
            back(0, s)
            if s < S - 1:
                fa_front(0, s + 1)
            fb(1, s)
        back(1, S - 1)

        loop_ctx.close()

        # ---------------- output head ----------------
        with tc.tile_pool(name="ops", bufs=2, space="PSUM") as ops, \
             tc.tile_pool(name="ost", bufs=2) as ost:
            for nq in range(8):
                op = ops.tile([OUT, 512], F32, tag="op")
                for kc in range(4):
                    mm(op[:], FCTS[:, kc * OUT:(kc + 1) * OUT],
                       TH[kc][:, nq * 512:(nq + 1) * 512],
                       start=(kc == 0), stop=False)
                mm(op[:], FCBS[:], ONES[:], start=False, stop=True)
                ot = ost.tile([OUT, 512], F32, tag="ot")
                nc.vector.tensor_copy(ot[:], op[:])
                nc.sync.dma_start(o[:, nq * 512:(nq + 1) * 512], ot[:])

    nc.compile()
    return nc


def _host_prep(inputs):
    f32 = np.float32
    ie = np.asarray(inputs["input_encoded"], f32)
    ys = np.asarray(inputs["y_seq"], f32)
    a1w = np.asarray(inputs["attn1_w"], f32)
    a1b = np.asarray(inputs["attn1_b"], f32)
    a2w = np.asarray(inputs["attn2_w"], f32)
    a2b = np.asarray(inputs["attn2_b"], f32)
    a3w = np.asarray(inputs["attn3_w"], f32)
    tw = np.asarray(inputs["tilde_w"], f32)
    tb = np.asarray(inputs["tilde_b"], f32)
    wih = np.asarray(inputs["w_ih"], f32)
    whh = np.asarray(inputs["w_hh"], f32)
    bih = np.asarray(inputs["b_ih"], f32)
    bhh = np.asarray(inputs["b_hh"], f32)
    f1w = np.asarray(inputs["fc1_w"], f32)
    f1b = np.asarray(inputs["fc1_b"], f32)
    f2w = np.asarray(inputs["fc2_w"], f32)
    f2b = np.asarray(inputs["fc2_b"], f32)

    wcomb = wih @ tw
    gbias = wih @ tb + bih + bhh
    fc = f2w @ f1w
    fcbias = f2w @ f1b + f2b

    perm = np.concatenate([np.arange(0, 512),
                           np.arange(768, 1024),
                           np.arange(512, 768)])
    wcombT = np.ascontiguousarray(wcomb.T)[:, perm]
    whhT = np.ascontiguousarray(whh.T)[:, perm]
    gbias_p = gbias[perm]
    wyb = np.concatenate([wcombT[0:3], gbias_p[None, :]], axis=0)

    # w3 diag arena: column ((b%16)&3) within each b's 32-col slice
    w3diag = np.zeros((128, 2, 32, 32), f32)
    for fc_ in range(2):
        for b in range(32):
            w3diag[:, fc_, b, (b % 16) & 3] = a3w[0, fc_ * 128:(fc_ + 1) * 128]
    w3diag = w3diag.reshape(128, 2048)

    # double the g-gate columns so one tanh(x/2) ACT call serves i,f,o,g
    wcombT[:, 768:1024] *= 2.0
    whhT[:, 768:1024] *= 2.0
    gbias_p = gbias_p.copy()
    gbias_p[768:1024] *= 2.0
    wyb = np.concatenate([wcombT[0:3], gbias_p[None, :]], axis=0)

    selm = np.zeros((128, 16), np.float32)
    for bp in range(16):
        selm[32 * (bp >> 2) + (bp & 3), bp] = 1.0

    fcT = np.ascontiguousarray(fc.T).copy()
    fcT[0:256] *= 0.5
    shared = {
        "w2t": np.ascontiguousarray(a2w.T).astype(BF16),
        "w1t": (np.ascontiguousarray(a1w.T) * 0.5).astype(BF16),
        "w3d": w3diag.astype(BF16),
        "bc": (a1b + a2b)[:, None].astype(f32),
        "wyb": wyb.astype(BF16),
        "wgc": np.ascontiguousarray(wcombT[3:259]).astype(BF16),
        "wgh": (whhT * 0.5).astype(BF16),
        "fct": fcT.astype(BF16),
        "fcb": fcbias[None, :].astype(BF16),
        "onesr": np.ones((1, 512), BF16),
        "i128": np.eye(128, dtype=f32),
        "selm": selm.astype(BF16),
    }
    in_maps = []
    for i in range(NCORES):
        b0 = i * BL
        xe = ie[b0:b0 + BL]                            # [32,128,256]
        m = dict(shared)
        # xt cols ordered (g, t, b'): group-contiguous 2048-col halves
        m["xt"] = np.ascontiguousarray(
            xe.reshape(2, GB, T, ENC).transpose(3, 0, 2, 1).reshape(ENC, BT)
        ).astype(BF16)
        m["x"] = xe.reshape(BT, ENC).astype(BF16)
        yt = ys[b0:b0 + BL].transpose(2, 1, 0).reshape(OUT, S * BL)
        m["yb"] = np.concatenate(
            [yt, np.ones((1, S * BL), f32)], axis=0).astype(BF16)
        in_maps.append(m)
    return in_maps


def kernel(**inputs):
    global _BUILT
    from concourse import bass_utils
    if _BUILT is None:
        _BUILT = _build_nc()
    nc = _BUILT
    import os
    in_maps = _host_prep(inputs)
    trace = bool(int(os.environ.get("KERNEL_TRACE", "0")))
    res = bass_utils.run_bass_kernel_spmd(nc, in_maps, core_ids=list(range(NCORES)),
                                          trace=trace)
    if trace:
        print(f"HW exec time: {res.exec_time_ns} ns  (mean {res.mean_exec_time_ns})")
        globals()['_LAST_RESULTS'] = res
    outs = []
    for i in range(NCORES):
        oc = res.results[i]["o"]                       # [3, 4096] (j, s*32+b)
        outs.append(oc.reshape(OUT, S, BL).transpose(2, 1, 0))
    return np.concatenate(outs, axis=0).astype(np.float32)


if __name__ == "__main__":
    pass

